# revision 21
# baseline (speedup 1.0000x reference)
"""Trainium2 Bass kernel for an ExcelFormer-style transformer block.

Reference computation (per batch item b of 32, seq 512, dim 256, 8 heads x 32):
    h   = LayerNorm(x) * gamma + beta
    qkv = h @ w_qkv ; causal attention with (sim + mask) * scale softmax
    out = attn_out @ w_out + b_out
    g   = out @ w_glu ; return out + g[:256] * tanh(g[256:])

Sharding: data-parallel over batch across 8 NeuronCores (4 batch items per
core). Each core runs the full block on its [4*512, 256] slice; no
collectives. Weights are replicated.

Per-core dataflow (activations kept transposed, [feature, token], so every
matmul streams from SBUF without extra transposes):
  - LN via bn_stats/bn_aggr, gamma/beta folded into the QKV weights on host
  - h transposed once via PE (32x [128,128] transposes)
  - qT/kT computed as W^T @ hT with heads packed 4-per-128-partitions;
    v computed untransposed (hT chunks as stationary operand)
  - QK^T row-tiled (4 heads concurrently in the 128x128 array, K=32 each),
    scores in [key j, query i] layout; causal handled by only computing
    the j<=i suffix; exp via one ACT op per j-chunk (scale folded in, no
    max-subtraction needed -- scores are O(1) and masked lanes underflow
    to exact 0); triangular mask on diagonal blocks via gpsimd.affine_select
  - P@V col-tiled (4 heads concurrent, V chunks stationary) accumulating
    attn_out^T in PSUM; rowsums via a col-tiled all-ones matmul
  - softmax division as attn^T * exp(-ln(rowsum)) (ACT ln/exp; DVE
    reciprocal is 8 cycles/elem and too slow)
  - out-proj consumes attn_out^T directly; GLU matmul uses po^T chunks as
    stationary and an extra identity-matmul to materialize the residual in
    untransposed [token, feature] layout for the final elementwise + DMA out
"""

import os

import numpy as np

import concourse.bass as bass
import concourse.mybir as mybir
import concourse.tile as tile
from concourse import bacc
from concourse.bass_utils import run_bass_kernel_spmd

F32 = mybir.dt.float32
F32R = mybir.dt.float32r
BF16 = mybir.dt.bfloat16
AF = mybir.ActivationFunctionType
ALU = mybir.AluOpType


B, S, DIM = 32, 512, 256
HEADS, DHEAD = 8, 32
INNER = HEADS * DHEAD
SCALE = DHEAD**-0.5
EPS = 1e-5

NCORES = 8
BC = B // NCORES  # batch items per core
T = BC * S  # tokens per core
P = 128
NT = T // P  # 16 token chunks of 128
KC = DIM // P  # 2 contraction chunks

LAST_RESULTS = None  # BassKernelResults of the most recent run (for test.py)


def _build_nc() -> bass.Bass:
    nc = bacc.Bacc(None, target_bir_lowering=False)

    x_d = nc.dram_tensor("x", [T, DIM], F32, kind="ExternalInput")
    wqk_d = nc.dram_tensor("w_qk", [DIM, 512], F32R, kind="ExternalInput")
    wv_d = nc.dram_tensor("w_v", [DIM, 256], F32R, kind="ExternalInput")
    wout_d = nc.dram_tensor("w_out", [DIM, 256], F32R, kind="ExternalInput")
    wglu_d = nc.dram_tensor("w_glu", [DIM, 512], F32R, kind="ExternalInput")
    bqk_d = nc.dram_tensor("b_qk", [P, 4], F32, kind="ExternalInput")
    bv_d = nc.dram_tensor("b_v", [1, 256], BF16, kind="ExternalInput")
    bout_d = nc.dram_tensor("b_out", [P, 2], F32, kind="ExternalInput")
    id_d = nc.dram_tensor("ident", [P, P], F32R, kind="ExternalInput")
    id32_d = nc.dram_tensor("ident32", [P, P], F32, kind="ExternalInput")
    ones_d = nc.dram_tensor("ones_in", [P, P], BF16, kind="ExternalInput")
    out_d = nc.dram_tensor("out", [T, DIM], F32, kind="ExternalOutput")

    with tile.TileContext(nc) as tc:
        with (
            tc.tile_pool(name="big", bufs=1) as big,
            tc.tile_pool(name="consts", bufs=1) as consts,
        ):
            # Persistent activations (per-partition KB noted)
            xin = big.tile([P, NT, DIM], F32)  # 16K
            hT = big.tile([P, KC, T], F32R)  # 16K
            qkT = big.tile([P, 4, T], F32R)  # 32K: [q0-3, q4-7, k0-3, k4-7]
            v_sb = big.tile([P, NT, 256], BF16)  # 16K
            otn = big.tile([P, KC, T], F32R)  # 16K normalized attn_out^T
            poT = big.tile([P, KC, T], F32R)  # 16K out-proj^T

            wqk = consts.tile([P, KC, 512], F32R)
            wv = consts.tile([P, KC, 256], F32R)
            wout = consts.tile([P, KC, 256], F32R)
            wglu = consts.tile([P, KC, 512], F32R)
            bqk = consts.tile([P, 4], F32)
            bv = consts.tile([1, 256], BF16)
            bout = consts.tile([P, 2], F32)
            ident = consts.tile([P, P], F32R)
            ident32 = consts.tile([P, P], F32)
            ones = consts.tile([P, P], BF16)

            for k in range(KC):
                nc.sync.dma_start(wqk[:, k, :], wqk_d[k * P : (k + 1) * P, :])
                nc.sync.dma_start(wv[:, k, :], wv_d[k * P : (k + 1) * P, :])
                nc.sync.dma_start(wout[:, k, :], wout_d[k * P : (k + 1) * P, :])
                nc.sync.dma_start(wglu[:, k, :], wglu_d[k * P : (k + 1) * P, :])
            nc.sync.dma_start(bqk[:], bqk_d[:])
            nc.sync.dma_start(bv[:], bv_d[:])
            nc.sync.dma_start(bout[:], bout_d[:])
            nc.sync.dma_start(ident[:], id_d[:])
            nc.sync.dma_start(ident32[:], id32_d[:])
            nc.sync.dma_start(ones[:], ones_d[:])

            # ---------------- LayerNorm + transpose ----------------
            with (
                tc.tile_pool(name="ln", bufs=4) as lnp,
                tc.tile_pool(name="lnstat", bufs=1) as lnstat,
                tc.tile_pool(
                    name="tps", bufs=4, space=bass.MemorySpace.PSUM
                ) as tps,
            ):
                mv_all = lnstat.tile([P, NT, 2], F32)
                std_all = lnstat.tile([P, NT], F32)
                rstd_all = lnstat.tile([P, NT], F32)
                nmr_all = lnstat.tile([P, NT], F32)
                eps_sb = lnstat.tile([P, 1], F32)
                nc.vector.memset(eps_sb[:], EPS)

                for i in range(NT):
                    nc.sync.dma_start(xin[:, i, :], x_d[i * P : (i + 1) * P, :])
                    st = lnp.tile([P, 6], F32, tag="st")
                    nc.vector.bn_stats(st[:], xin[:, i, :])
                    nc.vector.bn_aggr(mv_all[:, i, :], st[:])

                # std = sqrt(var + eps); rstd = 1/std; nmr = -mean * rstd
                nc.scalar.activation(
                    std_all[:], mv_all[:, :, 1], AF.Sqrt, bias=eps_sb[:], scale=1.0
                )
                nc.vector.reciprocal(rstd_all[:], std_all[:])
                nc.vector.scalar_tensor_tensor(
                    out=nmr_all[:],
                    in0=mv_all[:, :, 0],
                    scalar=-1.0,
                    in1=rstd_all[:],
                    op0=ALU.mult,
                    op1=ALU.mult,
                )

                for i in range(NT):
                    ht = lnp.tile([P, DIM], F32, tag="ht")
                    nc.vector.tensor_scalar(
                        out=ht[:],
                        in0=xin[:, i, :],
                        scalar1=rstd_all[:, i : i + 1],
                        scalar2=nmr_all[:, i : i + 1],
                        op0=ALU.mult,
                        op1=ALU.add,
                    )
                    for k in range(KC):
                        tp = tps.tile([P, P], F32, tag="tp")
                        nc.tensor.transpose(
                            tp[:], ht[:, k * P : (k + 1) * P], ident32[:]
                        )
                        dst = hT[:, k, i * P : (i + 1) * P]
                        if (i * KC + k) % 2 == 0:
                            nc.vector.tensor_copy(dst, tp[:])
                        else:
                            nc.scalar.copy(dst, tp[:])

            # ---------------- q^T/k^T and v projections ----------------
            with tc.tile_pool(
                name="mmps", bufs=4, space=bass.MemorySpace.PSUM
            ) as mmps:
                for m in range(4):  # feature chunks: q01, q23.. see layout
                    for n in range(4):  # token chunks of 512
                        ps = mmps.tile([P, 512], F32, tag="qk")
                        for k in range(KC):
                            nc.tensor.matmul(
                                ps[:],
                                wqk[:, k, m * P : (m + 1) * P],
                                hT[:, k, n * 512 : (n + 1) * 512],
                                start=(k == 0),
                                stop=(k == KC - 1),
                            )
                        dst = qkT[:, m, n * 512 : (n + 1) * 512]
                        if (m * 4 + n) % 2 == 0:
                            nc.vector.tensor_scalar(
                                out=dst,
                                in0=ps[:],
                                scalar1=bqk[:, m : m + 1],
                                scalar2=None,
                                op0=ALU.add,
                            )
                        else:
                            nc.scalar.activation(
                                dst, ps[:], AF.Identity,
                                bias=bqk[:, m : m + 1], scale=1.0,
                            )

                for i in range(NT):
                    ps = mmps.tile([P, 256], F32, tag="v")
                    for k in range(KC):
                        nc.tensor.matmul(
                            ps[:],
                            hT[:, k, i * P : (i + 1) * P],
                            wv[:, k, :],
                            start=(k == 0),
                            stop=False,
                        )
                    # + broadcast bias via K=1 ones matmul
                    nc.tensor.matmul(
                        ps[:], ones[0:1, :], bv[:], start=False, stop=True
                    )
                    if i % 2 == 0:
                        nc.vector.tensor_copy(v_sb[:, i, :], ps[:])
                    else:
                        nc.scalar.copy(v_sb[:, i, :], ps[:])

            # ---------------- attention ----------------
            with (
                tc.tile_pool(name="simp", bufs=1, space=bass.MemorySpace.PSUM) as simp,
                tc.tile_pool(name="otp", bufs=2, space=bass.MemorySpace.PSUM) as otp,
                tc.tile_pool(name="rsp", bufs=2, space=bass.MemorySpace.PSUM) as rsp,
                tc.tile_pool(name="ptp", bufs=3) as ptp,
                tc.tile_pool(name="attp", bufs=2) as attp,
            ):
                for b in range(BC):
                    for g in range(2):  # head groups 0-3, 4-7
                        t0 = b * S
                        sim = simp.tile([P, 4, 512], F32, tag="sim")
                        ot = otp.tile([P, 512], F32, tag="ot")
                        rs = rsp.tile([P, 512], F32, tag="rs")
                        for c in range(4):  # key chunks of 128
                            i0 = 128 * c
                            nct = 512 - i0  # causal: queries i >= j only
                            pt = ptp.tile([P, 4, 512], BF16, tag="pt")
                            for h in range(4):
                                nc.tensor.matmul(
                                    sim[:, h, 0:nct],
                                    qkT[32 * h : 32 * h + 32, 2 + g, t0 + i0 : t0 + i0 + P],
                                    qkT[32 * h : 32 * h + 32, g, t0 + i0 : t0 + 512],
                                    start=True,
                                    stop=True,
                                    tile_position=(32 * h, 0),
                                )
                            # exp((sim+mask)*scale): one ACT op over 4 heads
                            nc.scalar.activation(
                                pt[:, :, 0:nct], sim[:, :, 0:nct], AF.Exp,
                                bias=0.0, scale=SCALE,
                            )
                            # zero the strictly-disallowed (i<j) part of the
                            # diagonal 128x128 block, all 4 heads in one op
                            nc.gpsimd.affine_select(
                                out=pt[:, :, 0:P],
                                in_=pt[:, :, 0:P],
                                compare_op=ALU.is_ge,
                                fill=0.0,
                                base=0,
                                channel_multiplier=-1,
                                pattern=[[0, 4], [1, P]],
                            )
                            for h in range(4):
                                hg = 128 * g + 32 * h  # global head feature offset
                                nc.tensor.matmul(
                                    ot[32 * h : 32 * h + 32, i0:512],
                                    v_sb[:, b * 4 + c, hg : hg + 32],
                                    pt[:, h, 0:nct],
                                    start=(c == 0),
                                    stop=(c == 3),
                                    tile_position=(0, 32 * h),
                                )
                            for h in range(4):
                                nc.tensor.matmul(
                                    rs[32 * h : 32 * h + 32, i0:512],
                                    ones[:, 0:32],
                                    pt[:, h, 0:nct],
                                    start=(c == 0),
                                    stop=(c == 3),
                                    tile_position=(0, 32 * h),
                                )
                        # normalize: otn = ot * exp(-ln(rowsum))
                        lnr = attp.tile([P, 512], F32, tag="lnr")
                        nc.scalar.activation(lnr[:], rs[:], AF.Ln)
                        rrb = attp.tile([P, 512], F32, tag="rrb")
                        nc.scalar.activation(rrb[:], lnr[:], AF.Exp, scale=-1.0)
                        nc.vector.tensor_tensor(
                            otn[:, g, t0 : t0 + S], ot[:], rrb[:], ALU.mult
                        )

            # ---------------- out-projection (transposed) ----------------
            with tc.tile_pool(
                name="pops", bufs=4, space=bass.MemorySpace.PSUM
            ) as pops:
                for m in range(KC):
                    for n in range(4):
                        ps = pops.tile([P, 512], F32, tag="po")
                        for k in range(KC):
                            nc.tensor.matmul(
                                ps[:],
                                wout[:, k, m * P : (m + 1) * P],
                                otn[:, k, n * 512 : (n + 1) * 512],
                                start=(k == 0),
                                stop=(k == KC - 1),
                            )
                        dst = poT[:, m, n * 512 : (n + 1) * 512]
                        if (m * 4 + n) % 2 == 0:
                            nc.vector.tensor_scalar(
                                out=dst,
                                in0=ps[:],
                                scalar1=bout[:, m : m + 1],
                                scalar2=None,
                                op0=ALU.add,
                            )
                        else:
                            nc.scalar.activation(
                                dst, ps[:], AF.Identity,
                                bias=bout[:, m : m + 1], scale=1.0,
                            )

            # ---------------- GLU + residual + output ----------------
            with (
                tc.tile_pool(name="glups", bufs=3, space=bass.MemorySpace.PSUM) as glups,
                tc.tile_pool(name="glusb", bufs=3) as glusb,
            ):
                for i in range(NT):
                    gps = glups.tile([P, 768], F32, tag="g")
                    for k in range(KC):
                        nc.tensor.matmul(
                            gps[:, 0:512],
                            poT[:, k, i * P : (i + 1) * P],
                            wglu[:, k, :],
                            start=(k == 0),
                            stop=(k == KC - 1),
                        )
                    # identity matmul: materialize po (untransposed) for the
                    # residual in the same PSUM tile
                    for k in range(KC):
                        nc.tensor.matmul(
                            gps[:, 512 + k * P : 512 + (k + 1) * P],
                            poT[:, k, i * P : (i + 1) * P],
                            ident[:],
                            start=True,
                            stop=True,
                        )
                    th = glusb.tile([P, 256], F32, tag="th")
                    nc.scalar.activation(th[:], gps[:, 256:512], AF.Tanh)
                    prod = glusb.tile([P, 256], F32, tag="prod")
                    nc.vector.tensor_tensor(
                        prod[:], gps[:, 0:256], th[:], ALU.mult
                    )
                    res = glusb.tile([P, 256], F32, tag="res")
                    nc.vector.tensor_tensor(
                        res[:], gps[:, 512:768], prod[:], ALU.add
                    )
                    nc.sync.dma_start(out_d[i * P : (i + 1) * P, :], res[:])

    nc.compile()
    return nc


_NC_CACHE = None


def _get_nc():
    global _NC_CACHE
    if _NC_CACHE is None:
        _NC_CACHE = _build_nc()
    return _NC_CACHE


def prepare_in_maps(x, ln_gamma, ln_beta, w_qkv, w_out, b_out, w_glu):
    x = np.asarray(x, np.float32)
    ln_gamma = np.asarray(ln_gamma, np.float32)
    ln_beta = np.asarray(ln_beta, np.float32)
    w_qkv = np.asarray(w_qkv, np.float32)
    w_out = np.asarray(w_out, np.float32)
    b_out = np.asarray(b_out, np.float32)
    w_glu = np.asarray(w_glu, np.float32)

    # Fold LayerNorm affine into the QKV projection:
    #   (n*gamma + beta) @ W == n @ (gamma[:,None]*W) + beta @ W
    wq = ln_gamma[:, None] * w_qkv
    bq = ln_beta @ w_qkv  # [768]
    w_qk = np.ascontiguousarray(wq[:, : 2 * INNER])
    w_v = np.ascontiguousarray(wq[:, 2 * INNER :])
    b_qk = np.ascontiguousarray(bq[: 2 * INNER].reshape(4, P).T)  # [128, 4]
    import ml_dtypes
    b_v = np.ascontiguousarray(bq[2 * INNER :].reshape(1, 256)).astype(ml_dtypes.bfloat16)
    b_out_t = np.ascontiguousarray(b_out.reshape(2, P).T)  # [128, 2]
    ident = np.eye(P, dtype=np.float32)

    shared = {
        "w_qk": w_qk,
        "w_v": w_v,
        "w_out": np.ascontiguousarray(w_out),
        "w_glu": np.ascontiguousarray(w_glu),
        "b_qk": b_qk,
        "b_v": b_v,
        "b_out": b_out_t,
        "ident": ident,
        "ident32": ident,
        "ones_in": np.ones((P, P), ml_dtypes.bfloat16),
    }
    in_maps = []
    for c in range(NCORES):
        m = dict(shared)
        m["x"] = np.ascontiguousarray(
            x[c * BC : (c + 1) * BC].reshape(T, DIM)
        )
        in_maps.append(m)
    return in_maps


def kernel(x, ln_gamma, ln_beta, w_qkv, w_out, b_out, w_glu):
    global LAST_RESULTS
    in_maps = prepare_in_maps(x, ln_gamma, ln_beta, w_qkv, w_out, b_out, w_glu)
    nc = _get_nc()
    res = run_bass_kernel_spmd(
        nc,
        in_maps,
        core_ids=list(range(NCORES)),
        trace=bool(int(os.environ.get("KERNEL_TRACE", "0"))),
    )
    LAST_RESULTS = res
    out = np.concatenate(
        [r["out"].reshape(BC, S, DIM) for r in res.results], axis=0
    )
    return out.astype(np.float32)


def bench_exec_ns(in_maps, iters=16, reps=6):
    """Estimate per-execution HW time by chaining `iters` kernel executions
    inside a single jitted dispatch (axon RPC floor is ~100 ms, so a single
    execution cannot be wall-clocked). Sequencing is enforced with
    lax.optimization_barrier; per-iter time comes from the (iters vs 1) slope.

    Returns (ns_per_iter, details dict).
    """
    import time

    import jax
    from jax import lax
    from jax.sharding import Mesh, PartitionSpec
    from jax.experimental.shard_map import shard_map

    from concourse import bass2jax
    import concourse.mybir as mybir_

    nc = _get_nc()
    bass2jax.install_neuronx_cc_hook()
    pid_name = (
        nc.partition_id_tensor.name if nc.partition_id_tensor is not None else None
    )

    in_names, out_names, out_avals = [], [], []
    for alloc in nc.m.functions[0].allocations:
        if not isinstance(alloc, mybir_.MemoryLocationSet):
            continue
        name = alloc.memorylocations[0].name
        if alloc.kind == "ExternalInput":
            if name != pid_name:
                in_names.append(name)
        elif alloc.kind == "ExternalOutput":
            out_names.append(name)
            shape = tuple(alloc.tensor_shape)
            dt = mybir_.dt.np(alloc.dtype)
            out_avals.append(jax.core.ShapedArray(shape, dt))
    n_params = len(in_names)
    all_names = in_names + out_names
    if pid_name is not None:
        all_names = all_names + [pid_name]
    xi = in_names.index("x")

    def make_body(n_iter):
        # bass_exec is an (ordered) effectful primitive: N identical binds
        # are neither CSE'd nor reordered, and on one core they execute
        # back-to-back on the device queue. Operands must be raw jit
        # parameters (the neuronx_cc_hook rejects any other producing op).
        def _body(*args):
            extra = (
                [bass2jax.partition_id_tensor()] if pid_name is not None else []
            )
            out = None
            for _ in range(n_iter):
                outs = bass2jax._bass_exec_p.bind(
                    *args,
                    *extra,
                    out_avals=tuple(out_avals),
                    in_names=tuple(all_names),
                    out_names=tuple(out_names),
                    lowering_input_output_aliases=(),
                    sim_require_finite=True,
                    sim_require_nnan=True,
                    nc=nc,
                )
                out = outs[0]
            return (out,)

        return _body

    devices = jax.devices()[:NCORES]
    mesh = Mesh(np.asarray(devices), ("core",))
    per_core = [[np.asarray(m[name]) for name in in_names] for m in in_maps]
    concat_in = [
        np.concatenate([per_core[c][i] for c in range(NCORES)], axis=0)
        for i in range(n_params)
    ]
    concat_in += [
        np.zeros((NCORES * a.shape[0], *a.shape[1:]), a.dtype) for a in out_avals
    ]
    n_args = len(concat_in)
    from jax.sharding import NamedSharding
    sh = NamedSharding(mesh, PartitionSpec("core"))
    concat_in = [jax.device_put(a, sh) for a in concat_in]

    def timed(n_iter):
        fn = jax.jit(
            shard_map(
                make_body(n_iter),
                mesh=mesh,
                in_specs=(PartitionSpec("core"),) * n_args,
                out_specs=(PartitionSpec("core"),),
                check_rep=False,
            )
        )
        r = fn(*concat_in)[0]
        r.block_until_ready()  # compile + warm
        ts = []
        for _ in range(reps):
            t0 = time.perf_counter()
            r = fn(*concat_in)[0]
            r.block_until_ready()
            ts.append(time.perf_counter() - t0)
        return min(ts), np.asarray(r)

    t1, out1 = timed(1)
    tn, outn = timed(iters)
    ns = (tn - t1) / (iters - 1) * 1e9
    return ns, {
        "t1_s": t1,
        "tn_s": tn,
        "iters": iters,
        "out_check_diff": float(np.abs(out1 - outn).max()),
    }
